# revision 46
# baseline (speedup 1.0000x reference)
"""Maxwell viscoelastic model (linear recurrence scan) on 8 Trainium2 NeuronCores.

Math (per trajectory, T timesteps):
    a_n = 1 - k*dt_n                 (k = E/eta = 2)
    t_n = a_n*t_{n-1} + dt_n*eps_n   (t = gamma/k by linearity, t_0 = 0)
    sigma_n = 2.5*eps_n - 4*t_n

Strategy: batch (4096 trajectories) sharded across 8 cores (512 each).
All HBM traffic in bf16 (tolerance is 2e-2; measured pipeline error ~1%):
host deinterleaves x[:, :, 2] into eps/dt planes so every on-chip operand
is a dense step-1 bf16 vector.  Per core, 4 tiles of [128 x 4096], cut in
1024-step chunks that stream through a software pipeline:

  SYNC  per-chunk 512KB dt/eps loads (ring of 3 tiles, qSPDynamicHW);
        per-(slot,chunk) semaphores because completions can reorder
  ACT   a = 1 - 2*dt -> PSUM f32 (PSUM data0 keeps the DVE scan off the
        SBUF read port GpSimd shares — measured: scan(SBUF,SBUF)
        running beside a GpSimd op halves BOTH), per-chunk sigma
        PSUM->SBUF bf16 copies, output stores (qActDynamicHW)
  POOL  p = dt*eps -> SBUF bf16 (~2ns/elem, clean beside the scan)
  DVE   t = scan(a[PSUM], p[SBUF]) -> SBUF bf16: 2 cyc/elem serial
        feedback, the hard floor; plus the LAST chunk's sigma directly
        (GPS is idle by then so its 2-SBUF-read ops don't contend),
        which skips the PE+copy round trip on the drain path
  PE    sigma = -4*t + 2.5*eps: per chunk ldw(W4), mm halves -> PSUM
        start, ldw(W25), mm halves accumulate (diag weights from host).
        PE_HAM never unthrottles (the duty cycle is too bursty), so
        matmuls run at the cold 1.2 GHz rate — a few warmup matmuls
        overlap the fill in case the HAM phase is lucky

The scheduling trap this layout dodges: ACT executes in order, so a
sigma-copy that waits on a *recent* PE result would also block the next
a-pass and serialize the whole ring (scan->PE->copy->a->scan).  Each
sigma copy for chunk j is emitted three a-passes later (step j+3), by
which point PE(j) finished during scans j+1/j+2 — the stream becomes
feed-forward and the scan paces the kernel at ~2.5us/chunk.

Raw bass; every cross-engine and same-engine RAW goes through then_inc
completion counters (engine pipelines ack writes late).  PSUM exactly
full: a-chunks 2x4KB + sigma pair buffer 8KB.
"""

from contextlib import ExitStack

import numpy as np
import ml_dtypes

import concourse.bass as bass
import concourse.mybir as mybir
from concourse.bass_utils import run_bass_kernel_spmd

N_CORES = 8
P = 128                      # SBUF partitions
T_LEN = 4096                 # timesteps per trajectory
B_SHARD = 512                # trajectories per core
N_TILES = B_SHARD // P       # 4
CPT = 4                      # chunks per tile
CL = T_LEN // CPT            # 1024 chunk length
NQ = N_TILES * CPT           # 16 chunks per core
XT_BUFS = 4                  # input tile ring depth (all tiles resident)
TP_BUFS = 4                  # t/p slot ring depth
MM = 512                     # matmul moving-free max

BF16 = ml_dtypes.bfloat16


def build_nc() -> bass.Bass:
    nc = bass.Bass()
    f32 = mybir.dt.float32
    bf16 = mybir.dt.bfloat16
    mult = mybir.AluOpType.mult
    add = mybir.AluOpType.add
    Copy = mybir.ActivationFunctionType.Copy

    dt_d = nc.dram_tensor("dt", [B_SHARD, T_LEN], bf16, kind="ExternalInput")
    eps_d = nc.dram_tensor("eps", [B_SHARD, T_LEN], bf16, kind="ExternalInput")
    w4_d = nc.dram_tensor("w4", [P, P], bf16, kind="ExternalInput")
    w25_d = nc.dram_tensor("w25", [P, P], bf16, kind="ExternalInput")
    y_d = nc.dram_tensor("y", [B_SHARD, T_LEN], bf16, kind="ExternalOutput")

    dtr = dt_d.rearrange("(n p) t -> n p t", p=P)    # [4, 128, 4096]
    epr = eps_d.rearrange("(n p) t -> n p t", p=P)
    yr = y_d.rearrange("(n p) t -> n p t", p=P)

    def cs(c):
        return slice(c * CL, (c + 1) * CL)

    with ExitStack() as st:
        ec = st.enter_context
        dt_t = [ec(nc.sbuf_tensor(f"dt{s}", [P, T_LEN], bf16)) for s in range(XT_BUFS)]
        ep_t = [ec(nc.sbuf_tensor(f"ep{s}", [P, T_LEN], bf16)) for s in range(XT_BUFS)]
        t_t = [ec(nc.sbuf_tensor(f"t{s}", [P, CL], bf16)) for s in range(TP_BUFS)]
        p_t = [ec(nc.sbuf_tensor(f"p{s}", [P, CL], bf16)) for s in range(TP_BUFS)]
        sig = [ec(nc.sbuf_tensor(f"sig{s}", [P, T_LEN], bf16)) for s in range(2)]
        e25b = ec(nc.sbuf_tensor("e25b", [P, CL], bf16))   # last-chunk 2.5*eps
        g4b = ec(nc.sbuf_tensor("g4b", [P, CL], bf16))     # last-chunk -4*t
        sw4 = ec(nc.sbuf_tensor("sw4", [P, P], bf16))
        sw25 = ec(nc.sbuf_tensor("sw25", [P, P], bf16))
        pa = [ec(nc.psum_tensor(f"pa{s}", [P, CL], f32)) for s in range(2)]
        psig = ec(nc.psum_tensor("psig", [P, 2 * CL], f32))  # sigma pair buffer
        block = ec(nc.Block(no_gpsimd_drain=True))

        sem_xc = [[nc.alloc_semaphore(f"x{s}c{c}") for c in range(CPT)]
                  for s in range(XT_BUFS)]
        sem_w = nc.alloc_semaphore("w")
        act_a = nc.alloc_semaphore("act_a")    # +1 per a chunk
        act_cp = nc.alloc_semaphore("act_cp")  # +1 per sigma PAIR copy
        gps_p = nc.alloc_semaphore("gps_p")    # +1 per p chunk
        dve_s = nc.alloc_semaphore("dve_s")    # +1 per scan chunk
        pe_g = nc.alloc_semaphore("pe_g")      # +2 per chunk (mm half-groups)
        dve_sig = nc.alloc_semaphore("dve_sig")  # last-chunk sigma done
        act_e25 = nc.alloc_semaphore("act_e25")  # last-chunk e25 ready

        sem_out = [nc.alloc_semaphore(f"out{s}") for s in range(2)]

        # interleaved processing order: tiles paired (0,1) then (2,3); within
        # a pair chunks alternate tiles, so consecutive DVE scans belong to
        # DIFFERENT trajectories' chains — the init-RAW wait (scan q needs the
        # same tile's previous chunk = position q-2) resolves before scan q-1
        # even starts, hiding the ~150ns completion-sem round trip per chunk.
        order = []
        for pair in range(N_TILES // 2):
            for c in range(CPT):
                order.append((2 * pair, c))
                order.append((2 * pair + 1, c))

        # every (tile, chunk) load pair has its own semaphore: completions on
        # one queue can reorder, so a shared counter can't tell which chunk
        # landed; per-chunk gating also removes whole-tile completion cliffs.
        def x_wait(eng, i, c):
            eng.wait_ge(sem_xc[i % XT_BUFS][c], 32 * (i // XT_BUFS + 1))

        @block.sync
        def _(sync):
            for q, (i, c) in enumerate(order):
                sync.dma_start(
                    dt_t[i % XT_BUFS][:, cs(c)], dtr[i][:, cs(c)]
                ).then_inc(sem_xc[i % XT_BUFS][c], 16)
                sync.dma_start(
                    ep_t[i % XT_BUFS][:, cs(c)], epr[i][:, cs(c)]
                ).then_inc(sem_xc[i % XT_BUFS][c], 16)
                if q == 0:
                    sync.dma_start(sw4[:], w4_d[:, :]).then_inc(sem_w, 16)
                    sync.dma_start(sw25[:], w25_d[:, :]).then_inc(sem_w, 16)

        @block.gpsimd
        def _(gpsimd):
            for q, (i, c) in enumerate(order):
                x_wait(gpsimd, i, c)
                if q >= TP_BUFS:
                    # p slot WAR: scan(q-TP_BUFS) was the reader
                    gpsimd.wait_ge(dve_s, q - TP_BUFS + 1)
                gpsimd.tensor_tensor(
                    p_t[q % TP_BUFS][:], dt_t[i % XT_BUFS][:, cs(c)],
                    ep_t[i % XT_BUFS][:, cs(c)], mult,
                ).then_inc(gps_p, 1)

        @block.vector
        def _(vector):
            for q, (i, c) in enumerate(order):
                vector.wait_ge(act_a, q + 1)
                vector.wait_ge(gps_p, q + 1)
                if c != 0:
                    # same tile's previous chunk (position q-2) write-ack for
                    # the init read; already true once scan(q-2) completed
                    vector.wait_ge(dve_s, q - 1)
                if q >= TP_BUFS:
                    # t slot WAR: PE half-groups of chunk q-TP_BUFS done
                    vector.wait_ge(pe_g, 2 * (q - TP_BUFS + 1))
                init = 0.0 if c == 0 else t_t[(q - 2) % TP_BUFS][:, CL - 1:CL]
                vector.tensor_tensor_scan(
                    t_t[q % TP_BUFS][:], pa[q % 2][:], p_t[q % TP_BUFS][:],
                    init, mult, add,
                ).then_inc(dve_s, 1)
            # last chunk's sigma on DVE (GPS is idle by now, so the 2-SBUF-read
            # ops don't contend): skips the PE + PSUM-copy round trip at drain
            qL = NQ - 1
            vector.wait_ge(dve_s, NQ)
            vector.tensor_scalar(g4b[:], t_t[qL % TP_BUFS][:], -4.0, 0.0,
                                 mult, add).then_inc(dve_sig, 1)
            vector.wait_ge(act_e25, 1)
            vector.wait_ge(dve_sig, 1)
            vector.tensor_tensor(sig[(N_TILES - 1) % 2][:, cs(CPT - 1)],
                                 g4b[:], e25b[:], add).then_inc(dve_sig, 1)

        @block.tensor
        def _(pe):
            pe.wait_ge(sem_w, 32)
            # HAM warmup: the PE clock gate only opens (1.2 -> 2.4 GHz) after
            # ~3.4us of *sustained* matmul activity, and the per-chunk bursts
            # below never qualify, leaving every matmul at the cold rate.
            # Burn ~6us of back-to-back dummy matmuls on the (loaded) weight
            # tiles while the input DMAs fill the first chunks.
            for _ in range(24):
                pe.matmul(psig[:, :P], sw4[:], sw25[:], start=True, stop=True)
            for q, (i, c) in enumerate(order):
                if q == NQ - 1:
                    continue                   # last chunk: sigma on DVE
                half = (q % 2) * CL
                x_wait(pe, i, c)
                pe.wait_ge(dve_s, q + 1)       # t(q) ready
                if q >= 2:
                    # sigma half WAR: copy of chunk q-2 done
                    pe.wait_ge(act_cp, q - 1)
                # one ldweights per weight: W4 over both halves (PSUM
                # start), then W25 accumulating both halves
                for s in range(2):
                    sub = slice(half + s * MM, half + (s + 1) * MM)
                    tsub = slice(s * MM, (s + 1) * MM)
                    pe.matmul(psig[:, sub], sw4[:], t_t[q % TP_BUFS][:, tsub],
                              start=True, stop=False)
                for s in range(2):
                    sub = slice(half + s * MM, half + (s + 1) * MM)
                    esub = slice(c * CL + s * MM, c * CL + (s + 1) * MM)
                    pe.matmul(psig[:, sub], sw25[:],
                              ep_t[i % XT_BUFS][:, esub],
                              start=False, stop=True).then_inc(pe_g, 1)

        @block.scalar
        def _(scalar):
            def copy_chunk(j):
                ij, cj = order[j]
                if j == NQ - 1:
                    # last chunk: DVE wrote sigma to SBUF directly
                    scalar.wait_ge(dve_sig, 2)
                    scalar.dma_start(yr[ij][:, cs(cj)], sig[ij % 2][:, cs(cj)]
                                     ).then_inc(sem_out[ij % 2], 16)
                    return
                # sigma copy for chunk j (psig half j%2 -> sig bf16)
                scalar.wait_ge(pe_g, 2 * (j + 1))    # chunk j groups done
                if ij >= 2 and cj == 0:
                    scalar.wait_ge(sem_out[ij % 2], 16 * ((ij - 2) // 2 + 1))
                scalar.activation(
                    sig[ij % 2][:, cs(cj)], psig[:, (j % 2) * CL:(j % 2 + 1) * CL],
                    Copy,
                ).then_inc(act_cp, 1)
                if ij < N_TILES - 1:
                    if cj == CPT - 1:
                        # tile fully copied -> whole-tile store
                        scalar.wait_ge(act_cp, j + 1)
                        scalar.dma_start(yr[ij][:, :], sig[ij % 2][:, :]
                                         ).then_inc(sem_out[ij % 2], 16)
                else:
                    # last tile: store per chunk to shorten the drain
                    scalar.wait_ge(act_cp, j + 1)
                    scalar.dma_start(yr[ij][:, cs(cj)], sig[ij % 2][:, cs(cj)]
                                     ).then_inc(sem_out[ij % 2], 16)

            for step in range(NQ + 3):
                if step < NQ:
                    q = step
                    i, c = order[q]
                    x_wait(scalar, i, c)
                    if q >= 2:
                        # pa slot WAR: scan(q-2) read it
                        scalar.wait_ge(dve_s, q - 1)
                    scalar.activation(pa[q % 2][:], dt_t[i % XT_BUFS][:, cs(c)],
                                      Copy, bias=1.0, scale=-2.0
                                      ).then_inc(act_a, 1)
                    if q == NQ - 1:
                        # e25 for the DVE sigma path (input already waited)
                        scalar.activation(e25b[:],
                                          ep_t[i % XT_BUFS][:, cs(c)],
                                          Copy, bias=0.0, scale=2.5
                                          ).then_inc(act_e25, 1)
                # copy for chunk step-3: by then PE(step-3) finished during
                # scan(step-2)/(step-1), so ACT never stalls on a recent PE
                if step >= 3:
                    copy_chunk(step - 3)
            scalar.wait_ge(sem_out[0], 16 * 2)
            # slot1 stores: tile 1 whole + last-tile chunks (16 + 4*16)
            scalar.wait_ge(sem_out[1], 16 * 5)

    return nc


_NC_CACHE: dict = {}


def _get_nc() -> bass.Bass:
    if "nc" not in _NC_CACHE:
        _NC_CACHE["nc"] = build_nc()
    return _NC_CACHE["nc"]


def run(x: np.ndarray, trace: bool = False):
    """Run the sharded kernel; returns (full_output, BassKernelResults)."""
    b, t_len, ch = x.shape
    assert ch == 2 and b == N_CORES * B_SHARD and t_len == T_LEN
    x = np.asarray(x, dtype=np.float32)
    eps = np.ascontiguousarray(x[:, :, 0]).astype(BF16)
    dt = np.ascontiguousarray(x[:, :, 1]).astype(BF16)
    w4 = (np.eye(P, dtype=np.float32) * -4.0).astype(BF16)
    w25 = (np.eye(P, dtype=np.float32) * 2.5).astype(BF16)
    eps_sh = eps.reshape(N_CORES, B_SHARD, T_LEN)
    dt_sh = dt.reshape(N_CORES, B_SHARD, T_LEN)
    in_maps = [
        {"dt": dt_sh[i], "eps": eps_sh[i], "w4": w4, "w25": w25}
        for i in range(N_CORES)
    ]
    res = run_bass_kernel_spmd(
        _get_nc(), in_maps, core_ids=list(range(N_CORES)), trace=trace,
    )
    out = np.concatenate([r["y"].astype(np.float32) for r in res.results], axis=0)
    return out.reshape(b, t_len, 1), res


def kernel(x: np.ndarray) -> np.ndarray:
    out, _ = run(x, trace=False)
    return out


# revision 48
# speedup vs baseline: 1.0973x; 1.0973x over previous
"""Maxwell viscoelastic model (linear recurrence scan) on 8 Trainium2 NeuronCores.

Math (per trajectory, T timesteps):
    a_n = 1 - k*dt_n                 (k = E/eta = 2)
    t_n = a_n*t_{n-1} + dt_n*eps_n   (t = gamma/k by linearity, t_0 = 0)
    sigma_n = 2.5*eps_n - 4*t_n

Strategy: batch (4096 trajectories) sharded across 8 cores (512 each).
All HBM traffic in bf16 (tolerance is 2e-2; measured pipeline error ~1%):
host deinterleaves x[:, :, 2] into eps/dt planes so every on-chip operand
is a dense step-1 bf16 vector.  Per core, 4 tiles of [128 x 4096], cut in
1024-step chunks that stream through a software pipeline:

  SYNC  per-chunk 512KB dt/eps loads (ring of 3 tiles, qSPDynamicHW);
        per-(slot,chunk) semaphores because completions can reorder
  ACT   a = 1 - 2*dt -> PSUM f32 (PSUM data0 keeps the DVE scan off the
        SBUF read port GpSimd shares — measured: scan(SBUF,SBUF)
        running beside a GpSimd op halves BOTH), per-chunk sigma
        PSUM->SBUF bf16 copies, output stores (qActDynamicHW)
  POOL  p = dt*eps -> SBUF bf16 (~2ns/elem, clean beside the scan)
  DVE   t = scan(a[PSUM], p[SBUF]) -> SBUF bf16: 2 cyc/elem serial
        feedback, the hard floor; plus the LAST chunk's sigma directly
        (GPS is idle by then so its 2-SBUF-read ops don't contend),
        which skips the PE+copy round trip on the drain path
  PE    sigma = -4*t + 2.5*eps: per chunk ldw(W4), mm halves -> PSUM
        start, ldw(W25), mm halves accumulate (diag weights from host).
        PE_HAM never unthrottles (the duty cycle is too bursty), so
        matmuls run at the cold 1.2 GHz rate — a few warmup matmuls
        overlap the fill in case the HAM phase is lucky

The scheduling trap this layout dodges: ACT executes in order, so a
sigma-copy that waits on a *recent* PE result would also block the next
a-pass and serialize the whole ring (scan->PE->copy->a->scan).  Each
sigma copy for chunk j is emitted three a-passes later (step j+3), by
which point PE(j) finished during scans j+1/j+2 — the stream becomes
feed-forward and the scan paces the kernel at ~2.5us/chunk.

Raw bass; every cross-engine and same-engine RAW goes through then_inc
completion counters (engine pipelines ack writes late).  PSUM exactly
full: a-chunks 2x4KB + sigma pair buffer 8KB.
"""

from contextlib import ExitStack

import numpy as np
import ml_dtypes

import concourse.bass as bass
import concourse.mybir as mybir
from concourse.bass_utils import run_bass_kernel_spmd

N_CORES = 8
P = 128                      # SBUF partitions
T_LEN = 4096                 # timesteps per trajectory
B_SHARD = 512                # trajectories per core
N_TILES = B_SHARD // P       # 4
CPT = 4                      # chunks per tile
CL = T_LEN // CPT            # 1024 chunk length
NQ = N_TILES * CPT           # 16 chunks per core
XT_BUFS = 4                  # input tile ring depth (all tiles resident)
TP_BUFS = 4                  # t/p slot ring depth
MM = 512                     # matmul moving-free max

BF16 = ml_dtypes.bfloat16


def build_nc() -> bass.Bass:
    nc = bass.Bass()
    f32 = mybir.dt.float32
    bf16 = mybir.dt.bfloat16
    mult = mybir.AluOpType.mult
    add = mybir.AluOpType.add
    Copy = mybir.ActivationFunctionType.Copy

    dt_d = nc.dram_tensor("dt", [B_SHARD, T_LEN], bf16, kind="ExternalInput")
    eps_d = nc.dram_tensor("eps", [B_SHARD, T_LEN], bf16, kind="ExternalInput")
    w4_d = nc.dram_tensor("w4", [P, P], bf16, kind="ExternalInput")
    w25_d = nc.dram_tensor("w25", [P, P], bf16, kind="ExternalInput")
    y_d = nc.dram_tensor("y", [B_SHARD, T_LEN], bf16, kind="ExternalOutput")

    dtr = dt_d.rearrange("(n p) t -> n p t", p=P)    # [4, 128, 4096]
    epr = eps_d.rearrange("(n p) t -> n p t", p=P)
    yr = y_d.rearrange("(n p) t -> n p t", p=P)

    def cs(c):
        return slice(c * CL, (c + 1) * CL)

    with ExitStack() as st:
        ec = st.enter_context
        dt_t = [ec(nc.sbuf_tensor(f"dt{s}", [P, T_LEN], bf16)) for s in range(XT_BUFS)]
        ep_t = [ec(nc.sbuf_tensor(f"ep{s}", [P, T_LEN], bf16)) for s in range(XT_BUFS)]
        t_t = [ec(nc.sbuf_tensor(f"t{s}", [P, CL], bf16)) for s in range(TP_BUFS)]
        p_t = [ec(nc.sbuf_tensor(f"p{s}", [P, CL], bf16)) for s in range(TP_BUFS)]
        sig = [ec(nc.sbuf_tensor(f"sig{s}", [P, T_LEN], bf16)) for s in range(2)]
        e25b = ec(nc.sbuf_tensor("e25b", [P, CL], bf16))   # last-chunk 2.5*eps
        g4b = ec(nc.sbuf_tensor("g4b", [P, CL], bf16))     # last-chunk -4*t
        sw4 = ec(nc.sbuf_tensor("sw4", [P, P], bf16))
        sw25 = ec(nc.sbuf_tensor("sw25", [P, P], bf16))
        pa = [ec(nc.psum_tensor(f"pa{s}", [P, CL], f32)) for s in range(2)]
        psig = ec(nc.psum_tensor("psig", [P, 2 * CL], f32))  # sigma pair buffer
        block = ec(nc.Block(no_gpsimd_drain=True))

        sem_xc = [[nc.alloc_semaphore(f"x{s}c{c}") for c in range(CPT)]
                  for s in range(XT_BUFS)]
        sem_w = nc.alloc_semaphore("w")
        act_a = nc.alloc_semaphore("act_a")    # +1 per a chunk
        act_cp = nc.alloc_semaphore("act_cp")  # +1 per sigma PAIR copy
        gps_p = nc.alloc_semaphore("gps_p")    # +1 per p chunk
        dve_s = nc.alloc_semaphore("dve_s")    # +1 per scan chunk
        pe_g = nc.alloc_semaphore("pe_g")      # +2 per chunk (mm half-groups)
        dve_sig = nc.alloc_semaphore("dve_sig")  # last-chunk sigma done
        act_e25 = nc.alloc_semaphore("act_e25")  # last-chunk e25 ready

        sem_out = [nc.alloc_semaphore(f"out{s}") for s in range(2)]

        # interleaved processing order: tiles paired (0,1) then (2,3); within
        # a pair chunks alternate tiles, so consecutive DVE scans belong to
        # DIFFERENT trajectories' chains — the init-RAW wait (scan q needs the
        # same tile's previous chunk = position q-2) resolves before scan q-1
        # even starts, hiding the ~150ns completion-sem round trip per chunk.
        order = []
        for pair in range(N_TILES // 2):
            for c in range(CPT):
                order.append((2 * pair, c))
                order.append((2 * pair + 1, c))

        # every load transfer pair has its own semaphore: completions on one
        # queue can reorder, so a shared counter can't attribute them.
        def x_wait(eng, i, c):
            eng.wait_ge(x_sem(i, c), 32)

        @block.sync
        def _(sync):
            # first chunk of each pair-leading tile loads alone (fast pipeline
            # start), the rest in 512KB half-tile transfers (2 chunks each):
            # bigger transfers hold the SDMA near line rate so the tail chunks
            # land before the mid-kernel output stores contend for bandwidth.
            def load(i, c0, nch):
                span = slice(c0 * CL, (c0 + nch) * CL)
                sync.dma_start(dt_t[i][:, span], dtr[i][:, span]
                               ).then_inc(sem_xc[i][c0], 16)
                sync.dma_start(ep_t[i][:, span], epr[i][:, span]
                               ).then_inc(sem_xc[i][c0], 16)

            load(0, 0, 1)
            load(1, 0, 1)
            sync.dma_start(sw4[:], w4_d[:, :]).then_inc(sem_w, 16)
            sync.dma_start(sw25[:], w25_d[:, :]).then_inc(sem_w, 16)
            load(0, 1, 1)
            load(1, 1, 1)
            for pair, c0 in ((0, 2), (1, 0), (1, 2)):
                load(2 * pair, c0, 2)
                load(2 * pair + 1, c0, 2)

        # chunk (i, c) readiness: chunks 0/1 of tiles 0-1 have their own
        # semaphores; later chunks are covered by the half-tile transfer that
        # incremented sem_xc[i][c & ~1].
        def x_sem(i, c):
            if i < 2 and c < 2:
                return sem_xc[i][c]
            return sem_xc[i][c - (c % 2)]

        @block.gpsimd
        def _(gpsimd):
            for q, (i, c) in enumerate(order):
                x_wait(gpsimd, i, c)
                if q >= TP_BUFS:
                    # p slot WAR: scan(q-TP_BUFS) was the reader
                    gpsimd.wait_ge(dve_s, q - TP_BUFS + 1)
                gpsimd.tensor_tensor(
                    p_t[q % TP_BUFS][:], dt_t[i % XT_BUFS][:, cs(c)],
                    ep_t[i % XT_BUFS][:, cs(c)], mult,
                ).then_inc(gps_p, 1)

        @block.vector
        def _(vector):
            for q, (i, c) in enumerate(order):
                vector.wait_ge(act_a, q + 1)
                vector.wait_ge(gps_p, q + 1)
                if c != 0:
                    # same tile's previous chunk (position q-2) write-ack for
                    # the init read; already true once scan(q-2) completed
                    vector.wait_ge(dve_s, q - 1)
                if q >= TP_BUFS:
                    # t slot WAR: PE half-groups of chunk q-TP_BUFS done
                    vector.wait_ge(pe_g, 2 * (q - TP_BUFS + 1))
                init = 0.0 if c == 0 else t_t[(q - 2) % TP_BUFS][:, CL - 1:CL]
                vector.tensor_tensor_scan(
                    t_t[q % TP_BUFS][:], pa[q % 2][:], p_t[q % TP_BUFS][:],
                    init, mult, add,
                ).then_inc(dve_s, 1)
            # last chunk's sigma on DVE (GPS is idle by now, so the 2-SBUF-read
            # ops don't contend): skips the PE + PSUM-copy round trip at drain
            qL = NQ - 1
            vector.wait_ge(dve_s, NQ)
            vector.tensor_scalar(g4b[:], t_t[qL % TP_BUFS][:], -4.0, 0.0,
                                 mult, add).then_inc(dve_sig, 1)
            vector.wait_ge(act_e25, 1)
            vector.wait_ge(dve_sig, 1)
            vector.tensor_tensor(sig[(N_TILES - 1) % 2][:, cs(CPT - 1)],
                                 g4b[:], e25b[:], add).then_inc(dve_sig, 1)

        @block.tensor
        def _(pe):
            pe.wait_ge(sem_w, 32)
            # HAM warmup: the PE clock gate only opens (1.2 -> 2.4 GHz) after
            # ~3.4us of *sustained* matmul activity, and the per-chunk bursts
            # below never qualify, leaving every matmul at the cold rate.
            # Burn ~6us of back-to-back dummy matmuls on the (loaded) weight
            # tiles while the input DMAs fill the first chunks.
            for _ in range(24):
                pe.matmul(psig[:, :P], sw4[:], sw25[:], start=True, stop=True)
            for q, (i, c) in enumerate(order):
                if q == NQ - 1:
                    continue                   # last chunk: sigma on DVE
                half = (q % 2) * CL
                x_wait(pe, i, c)
                pe.wait_ge(dve_s, q + 1)       # t(q) ready
                if q >= 2:
                    # sigma half WAR: copy of chunk q-2 done
                    pe.wait_ge(act_cp, q - 1)
                # one ldweights per weight: W4 over both halves (PSUM
                # start), then W25 accumulating both halves
                for s in range(2):
                    sub = slice(half + s * MM, half + (s + 1) * MM)
                    tsub = slice(s * MM, (s + 1) * MM)
                    pe.matmul(psig[:, sub], sw4[:], t_t[q % TP_BUFS][:, tsub],
                              start=True, stop=False)
                for s in range(2):
                    sub = slice(half + s * MM, half + (s + 1) * MM)
                    esub = slice(c * CL + s * MM, c * CL + (s + 1) * MM)
                    pe.matmul(psig[:, sub], sw25[:],
                              ep_t[i % XT_BUFS][:, esub],
                              start=False, stop=True).then_inc(pe_g, 1)

        @block.scalar
        def _(scalar):
            def copy_chunk(j):
                ij, cj = order[j]
                if j == NQ - 1:
                    # last chunk: DVE wrote sigma to SBUF directly
                    scalar.wait_ge(dve_sig, 2)
                    scalar.dma_start(yr[ij][:, cs(cj)], sig[ij % 2][:, cs(cj)]
                                     ).then_inc(sem_out[ij % 2], 16)
                    return
                # sigma copy for chunk j (psig half j%2 -> sig bf16)
                scalar.wait_ge(pe_g, 2 * (j + 1))    # chunk j groups done
                if ij >= 2 and cj == 0:
                    scalar.wait_ge(sem_out[ij % 2], 16 * ((ij - 2) // 2 + 1))
                scalar.activation(
                    sig[ij % 2][:, cs(cj)], psig[:, (j % 2) * CL:(j % 2 + 1) * CL],
                    Copy,
                ).then_inc(act_cp, 1)
                if ij < N_TILES - 1:
                    if cj == CPT - 1:
                        # tile fully copied -> whole-tile store
                        scalar.wait_ge(act_cp, j + 1)
                        scalar.dma_start(yr[ij][:, :], sig[ij % 2][:, :]
                                         ).then_inc(sem_out[ij % 2], 16)
                else:
                    # last tile: store per chunk to shorten the drain
                    scalar.wait_ge(act_cp, j + 1)
                    scalar.dma_start(yr[ij][:, cs(cj)], sig[ij % 2][:, cs(cj)]
                                     ).then_inc(sem_out[ij % 2], 16)

            for step in range(NQ + 3):
                if step < NQ:
                    q = step
                    i, c = order[q]
                    x_wait(scalar, i, c)
                    if q >= 2:
                        # pa slot WAR: scan(q-2) read it
                        scalar.wait_ge(dve_s, q - 1)
                    scalar.activation(pa[q % 2][:], dt_t[i % XT_BUFS][:, cs(c)],
                                      Copy, bias=1.0, scale=-2.0
                                      ).then_inc(act_a, 1)
                    if q == NQ - 1:
                        # e25 for the DVE sigma path (input already waited)
                        scalar.activation(e25b[:],
                                          ep_t[i % XT_BUFS][:, cs(c)],
                                          Copy, bias=0.0, scale=2.5
                                          ).then_inc(act_e25, 1)
                # copy for chunk step-3: by then PE(step-3) finished during
                # scan(step-2)/(step-1), so ACT never stalls on a recent PE
                if step >= 3:
                    copy_chunk(step - 3)
            scalar.wait_ge(sem_out[0], 16 * 2)
            # slot1 stores: tile 1 whole + last-tile chunks (16 + 4*16)
            scalar.wait_ge(sem_out[1], 16 * 5)

    return nc


_NC_CACHE: dict = {}


def _get_nc() -> bass.Bass:
    if "nc" not in _NC_CACHE:
        _NC_CACHE["nc"] = build_nc()
    return _NC_CACHE["nc"]


def run(x: np.ndarray, trace: bool = False):
    """Run the sharded kernel; returns (full_output, BassKernelResults)."""
    b, t_len, ch = x.shape
    assert ch == 2 and b == N_CORES * B_SHARD and t_len == T_LEN
    x = np.asarray(x, dtype=np.float32)
    eps = np.ascontiguousarray(x[:, :, 0]).astype(BF16)
    dt = np.ascontiguousarray(x[:, :, 1]).astype(BF16)
    w4 = (np.eye(P, dtype=np.float32) * -4.0).astype(BF16)
    w25 = (np.eye(P, dtype=np.float32) * 2.5).astype(BF16)
    eps_sh = eps.reshape(N_CORES, B_SHARD, T_LEN)
    dt_sh = dt.reshape(N_CORES, B_SHARD, T_LEN)
    in_maps = [
        {"dt": dt_sh[i], "eps": eps_sh[i], "w4": w4, "w25": w25}
        for i in range(N_CORES)
    ]
    res = run_bass_kernel_spmd(
        _get_nc(), in_maps, core_ids=list(range(N_CORES)), trace=trace,
    )
    out = np.concatenate([r["y"].astype(np.float32) for r in res.results], axis=0)
    return out.reshape(b, t_len, 1), res


def kernel(x: np.ndarray) -> np.ndarray:
    out, _ = run(x, trace=False)
    return out


# revision 49
# speedup vs baseline: 1.1023x; 1.0045x over previous
"""Maxwell viscoelastic model (linear recurrence scan) on 8 Trainium2 NeuronCores.

Math (per trajectory, T timesteps):
    a_n = 1 - k*dt_n                 (k = E/eta = 2)
    t_n = a_n*t_{n-1} + dt_n*eps_n   (t = gamma/k by linearity, t_0 = 0)
    sigma_n = 2.5*eps_n - 4*t_n

Strategy: batch (4096 trajectories) sharded across 8 cores (512 each).
All HBM traffic in bf16 (tolerance is 2e-2; measured pipeline error ~1%):
host deinterleaves x[:, :, 2] into eps/dt planes so every on-chip operand
is a dense step-1 bf16 vector.  Per core, 4 tiles of [128 x 4096], cut in
1024-step chunks that stream through a software pipeline:

  SYNC  per-chunk 512KB dt/eps loads (ring of 3 tiles, qSPDynamicHW);
        per-(slot,chunk) semaphores because completions can reorder
  ACT   a = 1 - 2*dt -> PSUM f32 (PSUM data0 keeps the DVE scan off the
        SBUF read port GpSimd shares — measured: scan(SBUF,SBUF)
        running beside a GpSimd op halves BOTH), per-chunk sigma
        PSUM->SBUF bf16 copies, output stores (qActDynamicHW)
  POOL  p = dt*eps -> SBUF bf16 (~2ns/elem, clean beside the scan)
  DVE   t = scan(a[PSUM], p[SBUF]) -> SBUF bf16: 2 cyc/elem serial
        feedback, the hard floor; plus the LAST chunk's sigma directly
        (GPS is idle by then so its 2-SBUF-read ops don't contend),
        which skips the PE+copy round trip on the drain path
  PE    sigma = -4*t + 2.5*eps: per chunk ldw(W4), mm halves -> PSUM
        start, ldw(W25), mm halves accumulate (diag weights from host).
        PE_HAM never unthrottles (the duty cycle is too bursty), so
        matmuls run at the cold 1.2 GHz rate — a few warmup matmuls
        overlap the fill in case the HAM phase is lucky

The scheduling trap this layout dodges: ACT executes in order, so a
sigma-copy that waits on a *recent* PE result would also block the next
a-pass and serialize the whole ring (scan->PE->copy->a->scan).  Each
sigma copy for chunk j is emitted three a-passes later (step j+3), by
which point PE(j) finished during scans j+1/j+2 — the stream becomes
feed-forward and the scan paces the kernel at ~2.5us/chunk.

Raw bass; every cross-engine and same-engine RAW goes through then_inc
completion counters (engine pipelines ack writes late).  PSUM exactly
full: a-chunks 2x4KB + sigma pair buffer 8KB.
"""

from contextlib import ExitStack

import numpy as np
import ml_dtypes

import concourse.bass as bass
import concourse.mybir as mybir
from concourse.bass_utils import run_bass_kernel_spmd

N_CORES = 8
P = 128                      # SBUF partitions
T_LEN = 4096                 # timesteps per trajectory
B_SHARD = 512                # trajectories per core
N_TILES = B_SHARD // P       # 4
CPT = 4                      # chunks per tile
CL = T_LEN // CPT            # 1024 chunk length
NQ = N_TILES * CPT           # 16 chunks per core
XT_BUFS = 4                  # input tile ring depth (all tiles resident)
TP_BUFS = 4                  # t/p slot ring depth
MM = 512                     # matmul moving-free max

BF16 = ml_dtypes.bfloat16


def build_nc() -> bass.Bass:
    nc = bass.Bass()
    f32 = mybir.dt.float32
    bf16 = mybir.dt.bfloat16
    mult = mybir.AluOpType.mult
    add = mybir.AluOpType.add
    Copy = mybir.ActivationFunctionType.Copy

    dt_d = nc.dram_tensor("dt", [B_SHARD, T_LEN], bf16, kind="ExternalInput")
    eps_d = nc.dram_tensor("eps", [B_SHARD, T_LEN], bf16, kind="ExternalInput")
    w4_d = nc.dram_tensor("w4", [P, P], bf16, kind="ExternalInput")
    w25_d = nc.dram_tensor("w25", [P, P], bf16, kind="ExternalInput")
    y_d = nc.dram_tensor("y", [B_SHARD, T_LEN], bf16, kind="ExternalOutput")

    dtr = dt_d.rearrange("(n p) t -> n p t", p=P)    # [4, 128, 4096]
    epr = eps_d.rearrange("(n p) t -> n p t", p=P)
    yr = y_d.rearrange("(n p) t -> n p t", p=P)

    def cs(c):
        return slice(c * CL, (c + 1) * CL)

    with ExitStack() as st:
        ec = st.enter_context
        dt_t = [ec(nc.sbuf_tensor(f"dt{s}", [P, T_LEN], bf16)) for s in range(XT_BUFS)]
        ep_t = [ec(nc.sbuf_tensor(f"ep{s}", [P, T_LEN], bf16)) for s in range(XT_BUFS)]
        t_t = [ec(nc.sbuf_tensor(f"t{s}", [P, CL], bf16)) for s in range(TP_BUFS)]
        p_t = [ec(nc.sbuf_tensor(f"p{s}", [P, CL], bf16)) for s in range(TP_BUFS)]
        sig = [ec(nc.sbuf_tensor(f"sig{s}", [P, T_LEN], bf16)) for s in range(2)]
        e25b = ec(nc.sbuf_tensor("e25b", [P, CL], bf16))   # last-chunk 2.5*eps
        g4b = ec(nc.sbuf_tensor("g4b", [P, CL], bf16))     # last-chunk -4*t
        sw4 = ec(nc.sbuf_tensor("sw4", [P, P], bf16))
        sw25 = ec(nc.sbuf_tensor("sw25", [P, P], bf16))
        pa = [ec(nc.psum_tensor(f"pa{s}", [P, CL], f32)) for s in range(2)]
        psig = ec(nc.psum_tensor("psig", [P, 2 * CL], f32))  # sigma pair buffer
        block = ec(nc.Block(no_gpsimd_drain=True))

        sem_xc = [[nc.alloc_semaphore(f"x{s}c{c}") for c in range(CPT)]
                  for s in range(XT_BUFS)]
        sem_w = nc.alloc_semaphore("w")
        act_a = nc.alloc_semaphore("act_a")    # +1 per a chunk
        act_cp = nc.alloc_semaphore("act_cp")  # +1 per sigma PAIR copy
        gps_p = nc.alloc_semaphore("gps_p")    # +1 per p chunk
        dve_s = nc.alloc_semaphore("dve_s")    # +1 per scan chunk
        pe_g = nc.alloc_semaphore("pe_g")      # +2 per chunk (mm half-groups)
        dve_sig = nc.alloc_semaphore("dve_sig")  # last-chunk sigma done
        act_e25 = nc.alloc_semaphore("act_e25")  # last-chunk e25 ready

        sem_out = [nc.alloc_semaphore(f"out{s}") for s in range(2)]

        # interleaved processing order: tiles paired (0,1) then (2,3); within
        # a pair chunks alternate tiles, so consecutive DVE scans belong to
        # DIFFERENT trajectories' chains — the init-RAW wait (scan q needs the
        # same tile's previous chunk = position q-2) resolves before scan q-1
        # even starts, hiding the ~150ns completion-sem round trip per chunk.
        order = []
        for pair in range(N_TILES // 2):
            for c in range(CPT):
                order.append((2 * pair, c))
                order.append((2 * pair + 1, c))

        # every load transfer pair has its own semaphore: completions on one
        # queue can reorder, so a shared counter can't attribute them.
        def x_wait(eng, i, c):
            eng.wait_ge(x_sem(i, c), 32)

        @block.sync
        def _(sync):
            # first chunk of each pair-leading tile loads alone (fast pipeline
            # start), the rest in 512KB half-tile transfers (2 chunks each):
            # bigger transfers hold the SDMA near line rate so the tail chunks
            # land before the mid-kernel output stores contend for bandwidth.
            def load(i, c0, nch):
                span = slice(c0 * CL, (c0 + nch) * CL)
                sync.dma_start(dt_t[i][:, span], dtr[i][:, span]
                               ).then_inc(sem_xc[i][c0], 16)
                sync.dma_start(ep_t[i][:, span], epr[i][:, span]
                               ).then_inc(sem_xc[i][c0], 16)

            load(0, 0, 1)
            load(1, 0, 1)
            load(0, 1, 1)
            load(1, 1, 1)
            sync.dma_start(sw4[:], w4_d[:, :]).then_inc(sem_w, 16)
            sync.dma_start(sw25[:], w25_d[:, :]).then_inc(sem_w, 16)
            for pair, c0 in ((0, 2), (1, 0), (1, 2)):
                load(2 * pair, c0, 2)
                load(2 * pair + 1, c0, 2)

        # chunk (i, c) readiness: chunks 0/1 of tiles 0-1 have their own
        # semaphores; later chunks are covered by the half-tile transfer that
        # incremented sem_xc[i][c & ~1].
        def x_sem(i, c):
            if i < 2 and c < 2:
                return sem_xc[i][c]
            return sem_xc[i][c - (c % 2)]

        @block.gpsimd
        def _(gpsimd):
            for q, (i, c) in enumerate(order):
                x_wait(gpsimd, i, c)
                if q >= TP_BUFS:
                    # p slot WAR: scan(q-TP_BUFS) was the reader
                    gpsimd.wait_ge(dve_s, q - TP_BUFS + 1)
                gpsimd.tensor_tensor(
                    p_t[q % TP_BUFS][:], dt_t[i % XT_BUFS][:, cs(c)],
                    ep_t[i % XT_BUFS][:, cs(c)], mult,
                ).then_inc(gps_p, 1)

        @block.vector
        def _(vector):
            for q, (i, c) in enumerate(order):
                vector.wait_ge(act_a, q + 1)
                vector.wait_ge(gps_p, q + 1)
                if c != 0:
                    # same tile's previous chunk (position q-2) write-ack for
                    # the init read; already true once scan(q-2) completed
                    vector.wait_ge(dve_s, q - 1)
                if q >= TP_BUFS:
                    # t slot WAR: PE half-groups of chunk q-TP_BUFS done
                    vector.wait_ge(pe_g, 2 * (q - TP_BUFS + 1))
                init = 0.0 if c == 0 else t_t[(q - 2) % TP_BUFS][:, CL - 1:CL]
                vector.tensor_tensor_scan(
                    t_t[q % TP_BUFS][:], pa[q % 2][:], p_t[q % TP_BUFS][:],
                    init, mult, add,
                ).then_inc(dve_s, 1)
            # last chunk's sigma on DVE (GPS is idle by now, so the 2-SBUF-read
            # ops don't contend): skips the PE + PSUM-copy round trip at drain
            qL = NQ - 1
            vector.wait_ge(dve_s, NQ)
            vector.tensor_scalar(g4b[:], t_t[qL % TP_BUFS][:], -4.0, 0.0,
                                 mult, add).then_inc(dve_sig, 1)
            vector.wait_ge(act_e25, 1)
            vector.wait_ge(dve_sig, 1)
            vector.tensor_tensor(sig[(N_TILES - 1) % 2][:, cs(CPT - 1)],
                                 g4b[:], e25b[:], add).then_inc(dve_sig, 1)

        @block.tensor
        def _(pe):
            pe.wait_ge(sem_w, 32)
            # HAM warmup: the PE clock gate only opens (1.2 -> 2.4 GHz) after
            # ~3.4us of *sustained* matmul activity, and the per-chunk bursts
            # below never qualify, leaving every matmul at the cold rate.
            # Burn ~6us of back-to-back dummy matmuls on the (loaded) weight
            # tiles while the input DMAs fill the first chunks.
            for _ in range(24):
                pe.matmul(psig[:, :P], sw4[:], sw25[:], start=True, stop=True)
            for q, (i, c) in enumerate(order):
                if q == NQ - 1:
                    continue                   # last chunk: sigma on DVE
                half = (q % 2) * CL
                x_wait(pe, i, c)
                pe.wait_ge(dve_s, q + 1)       # t(q) ready
                if q >= 2:
                    # sigma half WAR: copy of chunk q-2 done
                    pe.wait_ge(act_cp, q - 1)
                # one ldweights per weight: W4 over both halves (PSUM
                # start), then W25 accumulating both halves
                for s in range(2):
                    sub = slice(half + s * MM, half + (s + 1) * MM)
                    tsub = slice(s * MM, (s + 1) * MM)
                    pe.matmul(psig[:, sub], sw4[:], t_t[q % TP_BUFS][:, tsub],
                              start=True, stop=False)
                for s in range(2):
                    sub = slice(half + s * MM, half + (s + 1) * MM)
                    esub = slice(c * CL + s * MM, c * CL + (s + 1) * MM)
                    pe.matmul(psig[:, sub], sw25[:],
                              ep_t[i % XT_BUFS][:, esub],
                              start=False, stop=True).then_inc(pe_g, 1)

        @block.scalar
        def _(scalar):
            def copy_chunk(j):
                ij, cj = order[j]
                if j == NQ - 1:
                    # last chunk: DVE wrote sigma to SBUF directly
                    scalar.wait_ge(dve_sig, 2)
                    scalar.dma_start(yr[ij][:, cs(cj)], sig[ij % 2][:, cs(cj)]
                                     ).then_inc(sem_out[ij % 2], 16)
                    return
                # sigma copy for chunk j (psig half j%2 -> sig bf16)
                scalar.wait_ge(pe_g, 2 * (j + 1))    # chunk j groups done
                if ij >= 2 and cj == 0:
                    scalar.wait_ge(sem_out[ij % 2], 16 * ((ij - 2) // 2 + 1))
                scalar.activation(
                    sig[ij % 2][:, cs(cj)], psig[:, (j % 2) * CL:(j % 2 + 1) * CL],
                    Copy,
                ).then_inc(act_cp, 1)
                if ij < N_TILES - 1:
                    if cj == CPT - 1:
                        # tile fully copied -> whole-tile store
                        scalar.wait_ge(act_cp, j + 1)
                        scalar.dma_start(yr[ij][:, :], sig[ij % 2][:, :]
                                         ).then_inc(sem_out[ij % 2], 16)
                else:
                    # last tile: store per chunk to shorten the drain
                    scalar.wait_ge(act_cp, j + 1)
                    scalar.dma_start(yr[ij][:, cs(cj)], sig[ij % 2][:, cs(cj)]
                                     ).then_inc(sem_out[ij % 2], 16)

            for step in range(NQ + 3):
                if step < NQ:
                    q = step
                    i, c = order[q]
                    x_wait(scalar, i, c)
                    if q >= 2:
                        # pa slot WAR: scan(q-2) read it
                        scalar.wait_ge(dve_s, q - 1)
                    scalar.activation(pa[q % 2][:], dt_t[i % XT_BUFS][:, cs(c)],
                                      Copy, bias=1.0, scale=-2.0
                                      ).then_inc(act_a, 1)
                    if q == NQ - 1:
                        # e25 for the DVE sigma path (input already waited)
                        scalar.activation(e25b[:],
                                          ep_t[i % XT_BUFS][:, cs(c)],
                                          Copy, bias=0.0, scale=2.5
                                          ).then_inc(act_e25, 1)
                # copy for chunk step-3: by then PE(step-3) finished during
                # scan(step-2)/(step-1), so ACT never stalls on a recent PE
                if step >= 3:
                    copy_chunk(step - 3)
            scalar.wait_ge(sem_out[0], 16 * 2)
            # slot1 stores: tile 1 whole + last-tile chunks (16 + 4*16)
            scalar.wait_ge(sem_out[1], 16 * 5)

    return nc


_NC_CACHE: dict = {}


def _get_nc() -> bass.Bass:
    if "nc" not in _NC_CACHE:
        _NC_CACHE["nc"] = build_nc()
    return _NC_CACHE["nc"]


def run(x: np.ndarray, trace: bool = False):
    """Run the sharded kernel; returns (full_output, BassKernelResults)."""
    b, t_len, ch = x.shape
    assert ch == 2 and b == N_CORES * B_SHARD and t_len == T_LEN
    x = np.asarray(x, dtype=np.float32)
    eps = np.ascontiguousarray(x[:, :, 0]).astype(BF16)
    dt = np.ascontiguousarray(x[:, :, 1]).astype(BF16)
    w4 = (np.eye(P, dtype=np.float32) * -4.0).astype(BF16)
    w25 = (np.eye(P, dtype=np.float32) * 2.5).astype(BF16)
    eps_sh = eps.reshape(N_CORES, B_SHARD, T_LEN)
    dt_sh = dt.reshape(N_CORES, B_SHARD, T_LEN)
    in_maps = [
        {"dt": dt_sh[i], "eps": eps_sh[i], "w4": w4, "w25": w25}
        for i in range(N_CORES)
    ]
    res = run_bass_kernel_spmd(
        _get_nc(), in_maps, core_ids=list(range(N_CORES)), trace=trace,
    )
    out = np.concatenate([r["y"].astype(np.float32) for r in res.results], axis=0)
    return out.reshape(b, t_len, 1), res


def kernel(x: np.ndarray) -> np.ndarray:
    out, _ = run(x, trace=False)
    return out


# revision 50
# speedup vs baseline: 1.1374x; 1.0319x over previous
"""Maxwell viscoelastic model (linear recurrence scan) on 8 Trainium2 NeuronCores.

Math (per trajectory, T timesteps):
    a_n = 1 - k*dt_n                 (k = E/eta = 2)
    t_n = a_n*t_{n-1} + dt_n*eps_n   (t = gamma/k by linearity, t_0 = 0)
    sigma_n = 2.5*eps_n - 4*t_n

Strategy: batch (4096 trajectories) sharded across 8 cores (512 each).
All HBM traffic in bf16 (tolerance is 2e-2; measured pipeline error ~1%):
host deinterleaves x[:, :, 2] into eps/dt planes so every on-chip operand
is a dense step-1 bf16 vector.  Per core, 4 tiles of [128 x 4096], cut in
1024-step chunks that stream through a software pipeline:

  SYNC  per-chunk 512KB dt/eps loads (ring of 3 tiles, qSPDynamicHW);
        per-(slot,chunk) semaphores because completions can reorder
  ACT   a = 1 - 2*dt -> PSUM f32 (PSUM data0 keeps the DVE scan off the
        SBUF read port GpSimd shares — measured: scan(SBUF,SBUF)
        running beside a GpSimd op halves BOTH), per-chunk sigma
        PSUM->SBUF bf16 copies, output stores (qActDynamicHW)
  POOL  p = dt*eps -> SBUF bf16 (~2ns/elem, clean beside the scan)
  DVE   t = scan(a[PSUM], p[SBUF]) -> SBUF bf16: 2 cyc/elem serial
        feedback, the hard floor; plus the LAST chunk's sigma directly
        (GPS is idle by then so its 2-SBUF-read ops don't contend),
        which skips the PE+copy round trip on the drain path
  PE    sigma = -4*t + 2.5*eps: per chunk ldw(W4), mm halves -> PSUM
        start, ldw(W25), mm halves accumulate (diag weights from host).
        PE_HAM never unthrottles (the duty cycle is too bursty), so
        matmuls run at the cold 1.2 GHz rate — a few warmup matmuls
        overlap the fill in case the HAM phase is lucky

The scheduling trap this layout dodges: ACT executes in order, so a
sigma-copy that waits on a *recent* PE result would also block the next
a-pass and serialize the whole ring (scan->PE->copy->a->scan).  Each
sigma copy for chunk j is emitted three a-passes later (step j+3), by
which point PE(j) finished during scans j+1/j+2 — the stream becomes
feed-forward and the scan paces the kernel at ~2.5us/chunk.

Raw bass; every cross-engine and same-engine RAW goes through then_inc
completion counters (engine pipelines ack writes late).  PSUM exactly
full: a-chunks 2x4KB + sigma pair buffer 8KB.
"""

from contextlib import ExitStack

import numpy as np
import ml_dtypes

import concourse.bass as bass
import concourse.mybir as mybir
from concourse.bass_utils import run_bass_kernel_spmd

N_CORES = 8
P = 128                      # SBUF partitions
T_LEN = 4096                 # timesteps per trajectory
B_SHARD = 512                # trajectories per core
N_TILES = B_SHARD // P       # 4
CPT = 4                      # chunks per tile
CL = T_LEN // CPT            # 1024 chunk length
NQ = N_TILES * CPT           # 16 chunks per core
XT_BUFS = 4                  # input tile ring depth (all tiles resident)
TP_BUFS = 4                  # t/p slot ring depth
MM = 512                     # matmul moving-free max

BF16 = ml_dtypes.bfloat16


def build_nc() -> bass.Bass:
    nc = bass.Bass()
    f32 = mybir.dt.float32
    bf16 = mybir.dt.bfloat16
    mult = mybir.AluOpType.mult
    add = mybir.AluOpType.add
    Copy = mybir.ActivationFunctionType.Copy

    dt_d = nc.dram_tensor("dt", [B_SHARD, T_LEN], bf16, kind="ExternalInput")
    eps_d = nc.dram_tensor("eps", [B_SHARD, T_LEN], bf16, kind="ExternalInput")
    w4_d = nc.dram_tensor("w4", [P, P], bf16, kind="ExternalInput")
    w25_d = nc.dram_tensor("w25", [P, P], bf16, kind="ExternalInput")
    y_d = nc.dram_tensor("y", [B_SHARD, T_LEN], bf16, kind="ExternalOutput")

    dtr = dt_d.rearrange("(n p) t -> n p t", p=P)    # [4, 128, 4096]
    epr = eps_d.rearrange("(n p) t -> n p t", p=P)
    yr = y_d.rearrange("(n p) t -> n p t", p=P)

    def cs(c):
        return slice(c * CL, (c + 1) * CL)

    with ExitStack() as st:
        ec = st.enter_context
        dt_t = [ec(nc.sbuf_tensor(f"dt{s}", [P, T_LEN], bf16)) for s in range(XT_BUFS)]
        ep_t = [ec(nc.sbuf_tensor(f"ep{s}", [P, T_LEN], bf16)) for s in range(XT_BUFS)]
        t_t = [ec(nc.sbuf_tensor(f"t{s}", [P, CL], bf16)) for s in range(TP_BUFS)]
        p_t = [ec(nc.sbuf_tensor(f"p{s}", [P, CL], bf16)) for s in range(TP_BUFS)]
        sig = [ec(nc.sbuf_tensor(f"sig{s}", [P, T_LEN], bf16)) for s in range(2)]
        e25b = ec(nc.sbuf_tensor("e25b", [P, CL], bf16))   # last-chunk 2.5*eps
        g4b = ec(nc.sbuf_tensor("g4b", [P, CL], bf16))     # last-chunk -4*t
        sw4 = ec(nc.sbuf_tensor("sw4", [P, P], bf16))
        sw25 = ec(nc.sbuf_tensor("sw25", [P, P], bf16))
        pa = [ec(nc.psum_tensor(f"pa{s}", [P, CL], f32)) for s in range(2)]
        psig = ec(nc.psum_tensor("psig", [P, 2 * CL], f32))  # sigma pair buffer
        block = ec(nc.Block(no_gpsimd_drain=True))

        sem_xc = [[nc.alloc_semaphore(f"x{s}c{c}") for c in range(CPT)]
                  for s in range(XT_BUFS)]
        sem_w = nc.alloc_semaphore("w")
        act_a = nc.alloc_semaphore("act_a")    # +1 per a chunk
        act_cp = nc.alloc_semaphore("act_cp")  # +1 per sigma PAIR copy
        gps_p = nc.alloc_semaphore("gps_p")    # +1 per p chunk
        dve_s = nc.alloc_semaphore("dve_s")    # +1 per scan chunk
        pe_g = nc.alloc_semaphore("pe_g")      # +2 per chunk (mm half-groups)
        dve_sig = nc.alloc_semaphore("dve_sig")  # last-chunk sigma done
        act_e25 = nc.alloc_semaphore("act_e25")  # last-chunk e25 ready

        sem_out = [nc.alloc_semaphore(f"out{s}") for s in range(2)]

        # every (tile, chunk) load pair has its own semaphore: completions on
        # one queue can reorder, so a shared counter can't tell which chunk
        # landed; per-chunk gating also removes whole-tile completion cliffs.
        def x_wait(eng, i, c):
            eng.wait_ge(sem_xc[i % XT_BUFS][c], 32 * (i // XT_BUFS + 1))

        @block.sync
        def _(sync):
            for i in range(N_TILES):
                for c in range(CPT):
                    q = CPT * i + c
                    if i >= XT_BUFS:
                        # slot chunk reuse: readers of (i-XT_BUFS, c) done
                        jq = CPT * (i - XT_BUFS) + c
                        sync.wait_ge(act_a, jq + 1)
                        sync.wait_ge(gps_p, jq + 1)
                        sync.wait_ge(pe_g, 2 * (jq + 1))
                    sync.dma_start(
                        dt_t[i % XT_BUFS][:, cs(c)], dtr[i][:, cs(c)]
                    ).then_inc(sem_xc[i % XT_BUFS][c], 16)
                    sync.dma_start(
                        ep_t[i % XT_BUFS][:, cs(c)], epr[i][:, cs(c)]
                    ).then_inc(sem_xc[i % XT_BUFS][c], 16)
                    if q == 0:
                        sync.dma_start(sw4[:], w4_d[:, :]).then_inc(sem_w, 16)
                        sync.dma_start(sw25[:], w25_d[:, :]).then_inc(sem_w, 16)

        @block.gpsimd
        def _(gpsimd):
            for i in range(N_TILES):
                for c in range(CPT):
                    q = CPT * i + c
                    x_wait(gpsimd, i, c)
                    if q >= TP_BUFS:
                        # p slot WAR: scan(q-TP_BUFS) was the reader
                        gpsimd.wait_ge(dve_s, q - TP_BUFS + 1)
                    gpsimd.tensor_tensor(
                        p_t[q % TP_BUFS][:], dt_t[i % XT_BUFS][:, cs(c)],
                        ep_t[i % XT_BUFS][:, cs(c)], mult,
                    ).then_inc(gps_p, 1)

        @block.vector
        def _(vector):
            for i in range(N_TILES):
                for c in range(CPT):
                    q = CPT * i + c
                    vector.wait_ge(act_a, q + 1)
                    vector.wait_ge(gps_p, q + 1)
                    if q >= 1:
                        # scan(q-1) complete: init RAW / t-slot WAR vs init
                        vector.wait_ge(dve_s, q)
                    if q >= TP_BUFS:
                        # t slot WAR: PE half-groups of chunk q-TP_BUFS done
                        vector.wait_ge(pe_g, 2 * (q - TP_BUFS + 1))
                    init = 0.0 if c == 0 else t_t[(q - 1) % TP_BUFS][:, CL - 1:CL]
                    vector.tensor_tensor_scan(
                        t_t[q % TP_BUFS][:], pa[q % 2][:], p_t[q % TP_BUFS][:],
                        init, mult, add,
                    ).then_inc(dve_s, 1)
            # last chunk's sigma on DVE (GPS is idle by now, so the 2-SBUF-read
            # ops don't contend): skips the PE + PSUM-copy round trip at drain
            qL = NQ - 1
            vector.wait_ge(dve_s, NQ)
            vector.tensor_scalar(g4b[:], t_t[qL % TP_BUFS][:], -4.0, 0.0,
                                 mult, add).then_inc(dve_sig, 1)
            vector.wait_ge(act_e25, 1)
            vector.wait_ge(dve_sig, 1)
            vector.tensor_tensor(sig[(N_TILES - 1) % 2][:, cs(CPT - 1)],
                                 g4b[:], e25b[:], add).then_inc(dve_sig, 1)

        @block.tensor
        def _(pe):
            pe.wait_ge(sem_w, 32)
            # HAM warmup: the PE clock gate only opens (1.2 -> 2.4 GHz) after
            # ~3.4us of *sustained* matmul activity, and the per-chunk bursts
            # below never qualify, leaving every matmul at the cold rate.
            # Burn ~6us of back-to-back dummy matmuls on the (loaded) weight
            # tiles while the input DMAs fill the first chunks.
            for _ in range(24):
                pe.matmul(psig[:, :P], sw4[:], sw25[:], start=True, stop=True)
            for i in range(N_TILES):
                for c in range(CPT):
                    q = CPT * i + c
                    if q == NQ - 1:
                        continue                   # last chunk: sigma on DVE
                    half = (q % 2) * CL
                    x_wait(pe, i, c)
                    pe.wait_ge(dve_s, q + 1)       # t(q) ready
                    if q >= 2:
                        # sigma half WAR: copy of chunk q-2 done
                        pe.wait_ge(act_cp, q - 1)
                    # one ldweights per weight: W4 over both halves (PSUM
                    # start), then W25 accumulating both halves
                    for s in range(2):
                        sub = slice(half + s * MM, half + (s + 1) * MM)
                        tsub = slice(s * MM, (s + 1) * MM)
                        pe.matmul(psig[:, sub], sw4[:], t_t[q % TP_BUFS][:, tsub],
                                  start=True, stop=False)
                    for s in range(2):
                        sub = slice(half + s * MM, half + (s + 1) * MM)
                        esub = slice(c * CL + s * MM, c * CL + (s + 1) * MM)
                        pe.matmul(psig[:, sub], sw25[:],
                                  ep_t[i % XT_BUFS][:, esub],
                                  start=False, stop=True).then_inc(pe_g, 1)

        @block.scalar
        def _(scalar):
            def copy_chunk(j):
                ij, cj = divmod(j, CPT)
                if j == NQ - 1:
                    # last chunk: DVE wrote sigma to SBUF directly
                    scalar.wait_ge(dve_sig, 2)
                    scalar.dma_start(yr[ij][:, cs(cj)], sig[ij % 2][:, cs(cj)]
                                     ).then_inc(sem_out[ij % 2], 16)
                    return
                # sigma copy for chunk j (psig half j%2 -> sig bf16)
                scalar.wait_ge(pe_g, 2 * (j + 1))    # chunk j groups done
                if ij >= 2 and cj == 0:
                    scalar.wait_ge(sem_out[ij % 2], 16 * ((ij - 2) // 2 + 1))
                scalar.activation(
                    sig[ij % 2][:, cs(cj)], psig[:, (j % 2) * CL:(j % 2 + 1) * CL],
                    Copy,
                ).then_inc(act_cp, 1)
                if ij < N_TILES - 1:
                    if cj == CPT - 1:
                        # tile fully copied -> whole-tile store
                        scalar.wait_ge(act_cp, j + 1)
                        scalar.dma_start(yr[ij][:, :], sig[ij % 2][:, :]
                                         ).then_inc(sem_out[ij % 2], 16)
                else:
                    # last tile: store per chunk to shorten the drain
                    scalar.wait_ge(act_cp, j + 1)
                    scalar.dma_start(yr[ij][:, cs(cj)], sig[ij % 2][:, cs(cj)]
                                     ).then_inc(sem_out[ij % 2], 16)

            for step in range(NQ + 3):
                if step < NQ:
                    q = step
                    i, c = divmod(q, CPT)
                    x_wait(scalar, i, c)
                    if q >= 2:
                        # pa slot WAR: scan(q-2) read it
                        scalar.wait_ge(dve_s, q - 1)
                    scalar.activation(pa[q % 2][:], dt_t[i % XT_BUFS][:, cs(c)],
                                      Copy, bias=1.0, scale=-2.0
                                      ).then_inc(act_a, 1)
                    if q == NQ - 1:
                        # e25 for the DVE sigma path (input already waited)
                        scalar.activation(e25b[:],
                                          ep_t[i % XT_BUFS][:, cs(c)],
                                          Copy, bias=0.0, scale=2.5
                                          ).then_inc(act_e25, 1)
                # copy for chunk step-3: by then PE(step-3) finished during
                # scan(step-2)/(step-1), so ACT never stalls on a recent PE
                if step >= 3:
                    copy_chunk(step - 3)
            scalar.wait_ge(sem_out[0], 16 * 2)
            # slot1 stores: tile 1 whole + last-tile chunks (16 + 4*16)
            scalar.wait_ge(sem_out[1], 16 * 5)

    return nc


_NC_CACHE: dict = {}


def _get_nc() -> bass.Bass:
    if "nc" not in _NC_CACHE:
        _NC_CACHE["nc"] = build_nc()
    return _NC_CACHE["nc"]


def run(x: np.ndarray, trace: bool = False):
    """Run the sharded kernel; returns (full_output, BassKernelResults)."""
    b, t_len, ch = x.shape
    assert ch == 2 and b == N_CORES * B_SHARD and t_len == T_LEN
    x = np.asarray(x, dtype=np.float32)
    eps = np.ascontiguousarray(x[:, :, 0]).astype(BF16)
    dt = np.ascontiguousarray(x[:, :, 1]).astype(BF16)
    w4 = (np.eye(P, dtype=np.float32) * -4.0).astype(BF16)
    w25 = (np.eye(P, dtype=np.float32) * 2.5).astype(BF16)
    eps_sh = eps.reshape(N_CORES, B_SHARD, T_LEN)
    dt_sh = dt.reshape(N_CORES, B_SHARD, T_LEN)
    in_maps = [
        {"dt": dt_sh[i], "eps": eps_sh[i], "w4": w4, "w25": w25}
        for i in range(N_CORES)
    ]
    res = run_bass_kernel_spmd(
        _get_nc(), in_maps, core_ids=list(range(N_CORES)), trace=trace,
    )
    out = np.concatenate([r["y"].astype(np.float32) for r in res.results], axis=0)
    return out.reshape(b, t_len, 1), res


def kernel(x: np.ndarray) -> np.ndarray:
    out, _ = run(x, trace=False)
    return out
